# revision 21
# baseline (speedup 1.0000x reference)
"""Trainium2 SPMD kernel for nn_AutoregressiveDecoder (gnn_message_passing).

Math (reference, per context g in 0..N-1, N=384):
    h1[g]  = concat(z, e_g) @ W1 = H0 + e_g (x) W1r     # H0 = z @ W1[:128]
    A[g]   = relu(P_g @ h1[g])         P_g = partials[g]
    h2[g]  = A[g] @ W2
    h3[g]  = P_g @ h2[g]
    S[g,:] = h3[g][g,:] @ h3[g].T      (row g of supplement, pre-tril)
    out    = x + 0.5*(tril(S) + tril(S).T)

8 cores x 48 slots, raw Bass (manual semaphores), all-bf16 matmuls.
Because the host applies tril, slot s only needs S[g, i] for i <= g.  Slots
map cyclically: slot s on core c handles context g = c + 8*(47-s), so one
SPMD program uses per-SLOT (core-independent) free sizes E_s = 8*(47-s)+10
for mm3/mm4 while staying load-balanced.  Per slot (pipelined, skew 3):
    mm1  A_T[h,:]  = sum_j H1g[j,h] Pt[j,:]                          N=384
    mm2  h2[j,k]   = sum_h A_T[h,j] W2[h,k]                          N=128
    mm3  h3T[k,:]  = sum_j h2[j,k] PtAug[j,0:E]                      N=E_s
    mm4  S[1,:]    = sum_k d[k] h3T[k,:]   (psum row aliased)        N=E_s
The rank-1 e_g (x) W1r term is folded into mm1's stationary operand: the
host pre-patches row g of the H0 chunk (in fp32) and ships the patched
[128, 256] chunk inside each slot's pt DMA, so no rank-1 matmuls and no
on-device fixup are needed.  PtAug chunk layout (W=388): cols 0,1 =
P_g[g, j] (so h3T col 0 = d, core-independent), cols 2..385 = Pt, pad --
all matmul slices stay 4-byte aligned.  TE semaphore waits sit where their
producers are provably a full iteration early (relu/S-row-drain before mm2,
h2-copy/h3-copy before mm3), keeping most matmul-to-matmul transitions free
of queue stalls.  ~38 garbage warm-up matmuls keep the PE HAM clock gate busy until
the startup DMAs (split across 3 queues) land, so real work starts at
2.4GHz.  tril/symmetrize/(+x) happen on host at unshard.
PE stream at iter i: mm1(i), mm2(i-1), mm3(i-2), mm4(i-3).
"""

import os
from contextlib import ExitStack

import numpy as np
import ml_dtypes

import concourse.bass as bass
import concourse.mybir as mybir
from concourse.bass_utils import run_bass_kernel_spmd

N = 384
D = 128
HID = 256
HID2 = 128
NCORES = 8
NB = N // NCORES  # 48 slots per core
W = N + 4  # pt chunk width: 2 dup prow cols + 384 Pt cols + 2 pad
PTBUF = 8  # pt SBUF ring depth
SRBUF = 8  # S-row SBUF ring depth
NWARM_BIG = 10  # N=384 garbage matmuls to pre-warm the PE HAM clock gate
NWARM_SMALL = 12  # N=128 fillers so the warm-up stream has no idle gap
PTW = 3 * W + HID  # pt row: 3 PtAug chunks + the patched H0 chunk

# per-slot mm3/mm4 free size: 2 prow cols + (g_max+1) needed cols, g_max =
# 8*(47-s)+7 -> E_s = 8*(47-s)+10  (even, <= 386)
ES = [8 * (NB - 1 - s) + 10 for s in range(NB)]
# chunk index / base row of the H0 row patched for slot s (g = 8*(47-s)+c)
TS = [(8 * (NB - 1 - s)) // 128 for s in range(NB)]
R0 = [(8 * (NB - 1 - s)) % 128 for s in range(NB)]

F32 = mybir.dt.float32
BF16 = mybir.dt.bfloat16
AFT = mybir.ActivationFunctionType

_NC_CACHE = {}
LAST_RESULT = None  # test.py reads exec_time_ns from here


def _pt_thr(slot: int) -> int:
    """sem_pt[slot % PTBUF] value after the fill for `slot` completes.

    Slot 0 is loaded by three chunk DMAs (3 x 16); all others by one.
    """
    n_fills = slot // PTBUF + 1
    return 16 * n_fills + (32 if slot % PTBUF == 0 else 0)


def _build_nc() -> bass.Bass:
    nc = bass.Bass()
    pt_d = nc.declare_dram_parameter("pt", [NB, 128, PTW], BF16, isOutput=False)
    h0f_d = nc.declare_dram_parameter("h0f", [128, 3 * HID], BF16, isOutput=False)
    w2f_d = nc.declare_dram_parameter("w2f", [128, 2 * HID2], BF16, isOutput=False)
    out_d = nc.declare_dram_parameter("outb", [1, NB * W], F32, isOutput=True)

    ctx = ExitStack()
    with ctx:
        # ---- persistent SBUF ----
        h0f = ctx.enter_context(nc.sbuf_tensor("h0f_s", [128, 3 * HID], BF16))
        w2f = ctx.enter_context(nc.sbuf_tensor("w2f_s", [128, 2 * HID2], BF16))
        pt = [
            ctx.enter_context(nc.sbuf_tensor(f"ptb{s}", [128, PTW], BF16))
            for s in range(PTBUF)
        ]
        at = [
            ctx.enter_context(nc.sbuf_tensor(f"atb{s}", [128, 2 * N], BF16))
            for s in range(3)
        ]
        h2sb = [
            ctx.enter_context(nc.sbuf_tensor(f"h2b{s}", [128, N], BF16))
            for s in range(3)
        ]
        h3sb = [
            ctx.enter_context(nc.sbuf_tensor(f"h3b{s}", [128, W], BF16))
            for s in range(3)
        ]
        srow = ctx.enter_context(nc.sbuf_tensor("srow_s", [1, SRBUF * W], F32))
        # ---- PSUM: 8 banks exactly ----
        aps = [
            [
                ctx.enter_context(
                    nc.psum_tensor(f"apsb{p}{h}", [128, N], F32)
                )
                for h in range(2)
            ]
            for p in range(2)
        ]  # aps[pair][hc]
        h2ps = [
            ctx.enter_context(nc.psum_tensor(f"h2psb{s}", [128, N], F32))
            for s in range(2)
        ]
        h3ps = [
            ctx.enter_context(nc.psum_tensor(f"h3psb{s}", [128, N + 2], F32))
            for s in range(2)
        ]

        # ---- semaphores ----
        sem_h0f = ctx.enter_context(nc.semaphore("sem_h0f"))
        sem_w2 = ctx.enter_context(nc.semaphore("sem_w2"))
        sem_pt = [
            ctx.enter_context(nc.semaphore(f"sem_pt{s}")) for s in range(PTBUF)
        ]
        sem_outb = [
            ctx.enter_context(nc.semaphore(f"sem_outb{s}")) for s in range(2)
        ]
        sem_mm1 = ctx.enter_context(nc.semaphore("sem_mm1"))
        sem_relu = ctx.enter_context(nc.semaphore("sem_relu"))
        sem_mm2 = ctx.enter_context(nc.semaphore("sem_mm2"))
        sem_h2c = ctx.enter_context(nc.semaphore("sem_h2c"))
        sem_mm3 = ctx.enter_context(nc.semaphore("sem_mm3"))
        sem_h3c = ctx.enter_context(nc.semaphore("sem_h3c"))
        sem_mm4 = ctx.enter_context(nc.semaphore("sem_mm4"))
        sem_sc = ctx.enter_context(nc.semaphore("sem_sc"))

        block = ctx.enter_context(nc.Block())

        NI = NB + 3  # pipeline iterations (skew 3)

        @block.sync
        def _(sync):
            sync.dma_start(h0f[:, 0 : 384], h0f_d[:, 0 : 384]).then_inc(
                sem_h0f, 16
            )
            sync.dma_start(h0f[:, 384 :], h0f_d[:, 384 :]).then_inc(sem_h0f, 16)
            for b in range(NB // 4):
                sync.wait_ge(sem_sc, 4 * b + 4)
                r = b % 2
                sync.dma_start(
                    out_d[:, 4 * b * W : (4 * b + 4) * W],
                    srow[:, 4 * r * W : (4 * r + 4) * W],
                ).then_inc(sem_outb[r], 16)

        @block.scalar
        def _(sc):
            sc.dma_start(pt[0][:, W : 2 * W], pt_d[0][:, W : 2 * W]).then_inc(
                sem_pt[0], 16
            )
            sc.dma_start(w2f[:, 0:HID2], w2f_d[:, 0:HID2]).then_inc(sem_w2, 16)
            sc.dma_start(w2f[:, HID2:], w2f_d[:, HID2:]).then_inc(sem_w2, 16)
            for i in range(NI):
                k = i
                if k < NB:
                    if k >= 3:
                        sc.wait_ge(sem_mm2, k - 2)  # at[k%3] reuse
                    for hc in range(2):
                        sc.wait_ge(sem_mm1, 2 * k + hc + 1)
                        nc.scalar.activation(
                            at[k % 3][:, hc * N : (hc + 1) * N],
                            aps[k % 2][hc][:, :],
                            AFT.Relu,
                        ).then_inc(sem_relu, 1)
                k = i - 3
                if 0 <= k < NB:
                    sc.wait_ge(sem_mm4, k + 1)
                    if k >= SRBUF:
                        # batch (k//4 - 2) drained the ring region we reuse
                        sc.wait_ge(sem_outb[(k // 4) % 2], 16 * (k // 8))
                    nc.scalar.activation(
                        srow[0:1, (k % SRBUF) * W : (k % SRBUF) * W + ES[k]],
                        h3ps[k % 2][0:1, 0 : ES[k]],
                        AFT.Copy,
                    ).then_inc(sem_sc, 1)

        @block.gpsimd
        def _(g):
            g.dma_start(pt[0][:, 0:W], pt_d[0][:, 0:W]).then_inc(sem_pt[0], 16)
            g.dma_start(pt[0][:, 2 * W :], pt_d[0][:, 2 * W :]).then_inc(
                sem_pt[0], 16
            )
            for p in range(1, min(PTBUF, NB)):
                if p >= 3:
                    # keep only ~2 prefetch DMAs in flight so early pt fills
                    # are not bandwidth-shared (rings interleave packets)
                    g.wait_ge(sem_pt[(p - 2) % PTBUF], _pt_thr(p - 2))
                g.dma_start(pt[p][:, :], pt_d[p]).then_inc(sem_pt[p], 16)
            for i in range(NI):
                p = i + PTBUF
                if p < NB:
                    g.wait_ge(sem_mm3, i + 1)
                    g.dma_start(
                        pt[p % PTBUF][:, :], pt_d[p]
                    ).then_inc(sem_pt[p % PTBUF], 16)

        @block.tensor
        def _(te):
            # ---- HAM warm-up: garbage matmuls while the startup DMAs run.
            # Data is whatever is in SBUF; aps[0][0] is overwritten by the
            # first real mm1 (start=True) before anything reads it. ----
            for _ in range(NWARM_BIG):
                nc.tensor.matmul(
                    aps[0][0][:, :],
                    h0f[:, 0:128],
                    pt[0][:, 2 : 2 + N],
                    start=True,
                    stop=True,
                    skip_group_check=True,
                )
            for _ in range(NWARM_SMALL):
                nc.tensor.matmul(
                    aps[0][0][:, 0:128],
                    h0f[:, 0:128],
                    pt[0][:, 2:130],
                    start=True,
                    stop=True,
                    skip_group_check=True,
                )
            te.wait_ge(sem_h0f, 32)
            for i in range(NI):
                if i == 1:
                    te.wait_ge(sem_w2, 32)
                if i == 0:
                    te.wait_ge(sem_pt[0], _pt_thr(0))
                k = i - 1
                if 0 <= k < NB:
                    # hoisted: relu(i-1) / S-row(i-4) land well before here
                    te.wait_ge(sem_relu, 2 * k + 2)
                    if k >= 3:
                        te.wait_ge(sem_sc, k - 2)

                # ---- mm1(i): A_T chunks, bf16 N=384; chunk TS[i] comes from
                # the DVE-patched scratch (H0 row g += W1r) ----
                if i < NB:
                    ptt = pt[i % PTBUF]
                    for hc in range(2):
                        for t in range(3):
                            if t == TS[i]:
                                lhsT = ptt[
                                    :, 3 * W + hc * 128 : 3 * W + hc * 128 + 128
                                ]
                            else:
                                lhsT = h0f[
                                    :, t * HID + hc * 128 : t * HID + hc * 128 + 128
                                ]
                            mm = nc.tensor.matmul(
                                aps[i % 2][hc][:, :],
                                lhsT,
                                ptt[:, t * W + 2 : t * W + 2 + N],
                                start=(t == 0),
                                stop=(t == 2),
                                skip_group_check=True,
                            )
                        # per-hc inc so ACT can start relu(hc0) while the
                        # hc1 matmuls still stream
                        mm.then_inc(sem_mm1, 1)

                # ---- mm2(i-1): h2 = A@W2, bf16 N=128 ----
                k = i - 1
                if 0 <= k < NB:
                    dst = h2ps[k % 2]
                    for jc in range(3):
                        for ht in range(2):
                            mm = nc.tensor.matmul(
                                dst[:, jc * 128 : (jc + 1) * 128],
                                at[k % 3][
                                    :, ht * N + jc * 128 : ht * N + jc * 128 + 128
                                ],
                                w2f[:, ht * HID2 : (ht + 1) * HID2],
                                start=(ht == 0),
                                stop=(ht == 1),
                            )
                    if not (0 <= i - 2 < NB):
                        mm.then_inc(sem_mm2, 1)  # no mm3 rider this iter
                # ---- mm3(i-2): h3T cols 0..E (col 0 = d), N=E_s ----
                k = i - 2
                if 0 <= k < NB:
                    E = ES[k]
                    te.wait_ge(sem_h2c, k + 1)
                    if k >= 1:
                        te.wait_ge(sem_h3c, k)  # h3sb for this iter's mm4
                    if not (0 <= i - 1 < NB) and k >= 2:
                        # drain iters: mm2 block above was absent
                        te.wait_ge(sem_sc, k - 1)
                    if i + 1 < NB:
                        # next iter's pt fill, checked early: the queue digests
                        # this while mm2/mm3 stream, keeping the mm4->mm1 gap
                        # free of wait instructions
                        te.wait_ge(sem_pt[(i + 1) % PTBUF], _pt_thr(i + 1))
                    dst = h3ps[k % 2]
                    ptt = pt[k % PTBUF]
                    for t in range(3):
                        mm = nc.tensor.matmul(
                            dst[:, 0:E],
                            h2sb[k % 3][:, t * 128 : (t + 1) * 128],
                            ptt[:, t * W : t * W + E],
                            start=(t == 0),
                            stop=(t == 2),
                        )
                        if t == 0 and k + 1 < NB:
                            # completion implies same-iter mm2(k+1) drained
                            mm.then_inc(sem_mm2, 1)
                    mm.then_inc(sem_mm3, 1)
                if not (0 <= i - 2 < NB) and i + 1 < NB:
                    # pipeline-fill iters: mm3 block above was absent
                    te.wait_ge(sem_pt[(i + 1) % PTBUF], _pt_thr(i + 1))
                # ---- mm4(i-3): S row into h3ps[k%2] partition 0 ----
                k = i - 3
                if 0 <= k < NB:
                    E = ES[k]
                    if not (0 <= i - 2 < NB):
                        # drain iters: mm3 block above was absent
                        te.wait_ge(sem_h3c, k + 1)
                    mm = nc.tensor.matmul(
                        h3ps[k % 2][0:1, 0:E],
                        h3sb[k % 3][:, 0:1],
                        h3sb[k % 3][:, 0:E],
                        start=True,
                        stop=True,
                    )
                    mm.then_inc(sem_mm4, 1)

        @block.vector
        def _(ve):
            for i in range(NI):
                k = i - 1
                if 0 <= k < NB:
                    if k >= 3:
                        ve.wait_ge(sem_mm3, k - 2)  # h2sb[k%3] reuse
                    ve.wait_ge(sem_mm2, k + 1)
                    nc.vector.tensor_copy(
                        h2sb[k % 3][:, :], h2ps[k % 2][:, :]
                    ).then_inc(sem_h2c, 1)
                k = i - 2
                if 0 <= k < NB:
                    if k >= 3:
                        ve.wait_ge(sem_mm4, k - 2)  # h3sb[k%3] reuse
                    ve.wait_ge(sem_mm3, k + 1)
                    nc.vector.tensor_copy(
                        h3sb[k % 3][:, 0 : ES[k]], h3ps[k % 2][:, 0 : ES[k]]
                    ).then_inc(sem_h3c, 1)

    return nc


def _get_nc() -> bass.Bass:
    if "nc" not in _NC_CACHE:
        _NC_CACHE["nc"] = _build_nc()
    return _NC_CACHE["nc"]


def kernel(z, x, partials, W1, W2):
    global LAST_RESULT
    z = np.asarray(z, dtype=np.float32)
    x = np.asarray(x, dtype=np.float32)
    partials = np.asarray(partials, dtype=np.float32)
    W1 = np.asarray(W1, dtype=np.float32)
    W2 = np.asarray(W2, dtype=np.float32)

    H0 = z[0] @ W1[:D]  # [384, 256]
    h0f = (
        np.ascontiguousarray(H0.reshape(3, 128, HID).transpose(1, 0, 2))
        .reshape(128, 3 * HID)
        .astype(ml_dtypes.bfloat16)
    )
    w2f = (
        np.ascontiguousarray(W2.reshape(2, 128, HID2).transpose(1, 0, 2))
        .reshape(128, 2 * HID2)
        .astype(ml_dtypes.bfloat16)
    )

    ptT = np.ascontiguousarray(partials.transpose(0, 2, 1))  # ptT[g,j,i]=P_g[i,j]
    ar = np.arange(N)
    prow = partials[ar, ar, :]  # [384, 384]  P_g[g, :]  (as fn of j)

    in_maps = []
    for c in range(NCORES):
        # slot s on core c handles context g = c + 8*(47-s)
        gs = np.array([c + NCORES * (NB - 1 - s) for s in range(NB)])
        aug = np.zeros((NB, 3, 128, W), dtype=ml_dtypes.bfloat16)
        aug[..., 2 : 2 + N] = ptT[gs].reshape(NB, 3, 128, N).astype(
            ml_dtypes.bfloat16
        )
        pr = prow[gs].reshape(NB, 3, 128).astype(ml_dtypes.bfloat16)
        aug[..., 0] = pr
        aug[..., 1] = pr
        aug = np.ascontiguousarray(aug.transpose(0, 2, 1, 3)).reshape(
            NB, 128, 3 * W
        )
        # patched H0 chunk per slot: row g of H1g = H0[g] + W1r, applied in
        # fp32 before the bf16 cast; shipped as the tail of the pt row
        pat = H0.reshape(3, 128, HID)[TS].copy()  # [NB, 128, HID] fp32
        pat[np.arange(NB), np.array(R0) + c, :] += W1[D]
        ptfull = np.concatenate(
            [aug, pat.astype(ml_dtypes.bfloat16)], axis=2
        )  # [NB, 128, PTW]
        in_maps.append(
            {
                "pt": np.ascontiguousarray(ptfull),
                "h0f": h0f,
                "w2f": w2f,
            }
        )

    nc = _get_nc()
    res = run_bass_kernel_spmd(
        nc,
        in_maps,
        core_ids=list(range(NCORES)),
        trace=bool(os.environ.get("KERNEL_TRACE")),
    )
    LAST_RESULT = res
    S = np.zeros((N, N), dtype=np.float32)
    for c in range(NCORES):
        rows = np.asarray(res.results[c]["outb"], np.float32).reshape(NB, W)
        for s in range(NB):
            g = c + NCORES * (NB - 1 - s)
            S[g, 0 : g + 1] = rows[s, 2 : 3 + g]
    sup = S + S.T  # S is already lower-triangular (diag doubled, as in TF)
    sup = sup * np.float32(0.5)
    return (x + sup).astype(np.float32)


# revision 22
# speedup vs baseline: 1.5618x; 1.5618x over previous
"""Trainium2 SPMD kernel for nn_AutoregressiveDecoder (gnn_message_passing).

Math (reference, per context g in 0..N-1, N=384):
    h1[g]  = concat(z, e_g) @ W1 = H0 + e_g (x) W1r     # H0 = z @ W1[:128]
    A[g]   = relu(P_g @ h1[g])         P_g = partials[g]
    h2[g]  = A[g] @ W2
    h3[g]  = P_g @ h2[g]
    S[g,:] = h3[g][g,:] @ h3[g].T      (row g of supplement, pre-tril)
    out    = x + 0.5*(tril(S) + tril(S).T)

8 cores x 48 slots, raw Bass (manual semaphores), all-bf16 matmuls.
Because the host applies tril, slot s only needs S[g, i] for i <= g.  Slots
map cyclically: slot s on core c handles context g = c + 8*(47-s), so one
SPMD program uses per-SLOT (core-independent) free sizes E_s = 8*(47-s)+10
for mm3/mm4 while staying load-balanced.  Per slot (pipelined, skew 3):
    mm1  A_T[h,:]  = sum_j H1g[j,h] Pt[j,:]                          N=384
    mm2  h2[j,k]   = sum_h A_T[h,j] W2[h,k]                          N=128
    mm3  h3T[k,:]  = sum_j h2[j,k] PtAug[j,0:E]                      N=E_s
    mm4  S[1,:]    = sum_k d[k] h3T[k,:]   (psum row aliased)        N=E_s
The rank-1 e_g (x) W1r term is folded into mm1's stationary operand: the
host pre-patches row g of the H0 chunk (in fp32) and ships the patched
[128, 256] chunk inside each slot's pt DMA, so no rank-1 matmuls and no
on-device fixup are needed.  PtAug chunk layout (W=388): cols 0,1 =
P_g[g, j] (so h3T col 0 = d, core-independent), cols 2..385 = Pt, pad --
all matmul slices stay 4-byte aligned.  TE semaphore waits sit where their
producers are provably a full iteration early (relu/S-row-drain before mm2,
h2-copy/h3-copy before mm3), keeping most matmul-to-matmul transitions free
of queue stalls.  ~38 garbage warm-up matmuls keep the PE HAM clock gate busy until
the startup DMAs (split across 3 queues) land, so real work starts at
2.4GHz.  tril/symmetrize/(+x) happen on host at unshard.
PE stream at iter i: mm1(i), mm2(i-1), mm3(i-2), mm4(i-3).
"""

import os
from contextlib import ExitStack

import numpy as np
import ml_dtypes

import concourse.bass as bass
import concourse.mybir as mybir
from concourse.bass_utils import run_bass_kernel_spmd

N = 384
D = 128
HID = 256
HID2 = 128
NCORES = 8
NB = N // NCORES  # 48 slots per core
W = N + 4  # pt chunk width: 2 dup prow cols + 384 Pt cols + 2 pad
PTBUF = 8  # pt SBUF ring depth
SRBUF = 8  # S-row SBUF ring depth
NWARM_BIG = 10  # N=384 garbage matmuls to pre-warm the PE HAM clock gate
NWARM_SMALL = 12  # N=128 fillers so the warm-up stream has no idle gap
PTW = 3 * W + HID  # pt row: 3 PtAug chunks + the patched H0 chunk

# per-slot mm3/mm4 free size: 2 prow cols + (g_max+1) needed cols, g_max =
# 8*(47-s)+7 -> E_s = 8*(47-s)+10  (even, <= 386)
ES = [8 * (NB - 1 - s) + 10 for s in range(NB)]
# chunk index / base row of the H0 row patched for slot s (g = 8*(47-s)+c)
TS = [(8 * (NB - 1 - s)) // 128 for s in range(NB)]
R0 = [(8 * (NB - 1 - s)) % 128 for s in range(NB)]

F32 = mybir.dt.float32
BF16 = mybir.dt.bfloat16
AFT = mybir.ActivationFunctionType

_NC_CACHE = {}
LAST_RESULT = None  # test.py reads exec_time_ns from here


def _pt_thr(slot: int) -> int:
    """sem_pt[slot % PTBUF] value after the fill for `slot` completes.

    Slot 0 is loaded by three chunk DMAs (3 x 16); all others by one.
    """
    n_fills = slot // PTBUF + 1
    return 16 * n_fills + (32 if slot % PTBUF == 0 else 0)


def _build_nc() -> bass.Bass:
    nc = bass.Bass()
    pt_d = nc.declare_dram_parameter("pt", [NB, 128, PTW], BF16, isOutput=False)
    h0f_d = nc.declare_dram_parameter("h0f", [128, 3 * HID], BF16, isOutput=False)
    w2f_d = nc.declare_dram_parameter("w2f", [128, 2 * HID2], BF16, isOutput=False)
    out_d = nc.declare_dram_parameter("outb", [1, NB * W], F32, isOutput=True)

    ctx = ExitStack()
    with ctx:
        # ---- persistent SBUF ----
        h0f = ctx.enter_context(nc.sbuf_tensor("h0f_s", [128, 3 * HID], BF16))
        w2f = ctx.enter_context(nc.sbuf_tensor("w2f_s", [128, 2 * HID2], BF16))
        pt = [
            ctx.enter_context(nc.sbuf_tensor(f"ptb{s}", [128, PTW], BF16))
            for s in range(PTBUF)
        ]
        at = [
            ctx.enter_context(nc.sbuf_tensor(f"atb{s}", [128, 2 * N], BF16))
            for s in range(3)
        ]
        h2sb = [
            ctx.enter_context(nc.sbuf_tensor(f"h2b{s}", [128, N], BF16))
            for s in range(3)
        ]
        h3sb = [
            ctx.enter_context(nc.sbuf_tensor(f"h3b{s}", [128, W], BF16))
            for s in range(3)
        ]
        srow = ctx.enter_context(nc.sbuf_tensor("srow_s", [1, SRBUF * W], F32))
        # ---- PSUM: 8 banks exactly ----
        aps = [
            [
                ctx.enter_context(
                    nc.psum_tensor(f"apsb{p}{h}", [128, N], F32)
                )
                for h in range(2)
            ]
            for p in range(2)
        ]  # aps[pair][hc]
        h2ps = [
            ctx.enter_context(nc.psum_tensor(f"h2psb{s}", [128, N], F32))
            for s in range(2)
        ]
        h3ps = [
            ctx.enter_context(nc.psum_tensor(f"h3psb{s}", [128, N + 2], F32))
            for s in range(2)
        ]

        # ---- semaphores ----
        sem_h0f = ctx.enter_context(nc.semaphore("sem_h0f"))
        sem_w2 = ctx.enter_context(nc.semaphore("sem_w2"))
        sem_pt = [
            ctx.enter_context(nc.semaphore(f"sem_pt{s}")) for s in range(PTBUF)
        ]
        sem_outb = [
            ctx.enter_context(nc.semaphore(f"sem_outb{s}")) for s in range(2)
        ]
        sem_mm1 = ctx.enter_context(nc.semaphore("sem_mm1"))
        sem_relu = ctx.enter_context(nc.semaphore("sem_relu"))
        sem_mm2 = ctx.enter_context(nc.semaphore("sem_mm2"))
        sem_h2c = ctx.enter_context(nc.semaphore("sem_h2c"))
        sem_mm3 = ctx.enter_context(nc.semaphore("sem_mm3"))
        sem_h3c = ctx.enter_context(nc.semaphore("sem_h3c"))
        sem_mm4 = ctx.enter_context(nc.semaphore("sem_mm4"))
        sem_sc = ctx.enter_context(nc.semaphore("sem_sc"))

        block = ctx.enter_context(nc.Block())

        NI = NB + 3  # pipeline iterations (skew 3)

        @block.sync
        def _(sync):
            sync.dma_start(h0f[:, 0 : 384], h0f_d[:, 0 : 384]).then_inc(
                sem_h0f, 16
            )
            sync.dma_start(h0f[:, 384 :], h0f_d[:, 384 :]).then_inc(sem_h0f, 16)
            for b in range(NB // 4):
                sync.wait_ge(sem_sc, 4 * b + 4)
                r = b % 2
                sync.dma_start(
                    out_d[:, 4 * b * W : (4 * b + 4) * W],
                    srow[:, 4 * r * W : (4 * r + 4) * W],
                ).then_inc(sem_outb[r], 16)

        @block.scalar
        def _(sc):
            sc.dma_start(pt[0][:, W : 2 * W], pt_d[0][:, W : 2 * W]).then_inc(
                sem_pt[0], 16
            )
            sc.dma_start(w2f[:, 0:HID2], w2f_d[:, 0:HID2]).then_inc(sem_w2, 16)
            sc.dma_start(w2f[:, HID2:], w2f_d[:, HID2:]).then_inc(sem_w2, 16)
            for i in range(NI):
                k = i
                if k < NB:
                    if k >= 3:
                        sc.wait_ge(sem_mm2, k - 2)  # at[k%3] reuse
                    for hc in range(2):
                        sc.wait_ge(sem_mm1, 2 * k + hc + 1)
                        nc.scalar.activation(
                            at[k % 3][:, hc * N : (hc + 1) * N],
                            aps[k % 2][hc][:, :],
                            AFT.Relu,
                        ).then_inc(sem_relu, 1)
                k = i - 3
                if 0 <= k < NB:
                    sc.wait_ge(sem_mm4, k + 1)
                    if k >= SRBUF:
                        # batch (k//4 - 2) drained the ring region we reuse
                        sc.wait_ge(sem_outb[(k // 4) % 2], 16 * (k // 8))
                    nc.scalar.activation(
                        srow[0:1, (k % SRBUF) * W : (k % SRBUF) * W + ES[k]],
                        h3ps[k % 2][0:1, 0 : ES[k]],
                        AFT.Copy,
                    ).then_inc(sem_sc, 1)

        @block.gpsimd
        def _(g):
            g.dma_start(pt[0][:, 0:W], pt_d[0][:, 0:W]).then_inc(sem_pt[0], 16)
            g.dma_start(pt[0][:, 2 * W :], pt_d[0][:, 2 * W :]).then_inc(
                sem_pt[0], 16
            )
            for p in range(1, min(PTBUF, NB)):
                if p >= 3:
                    # keep only ~2 prefetch DMAs in flight so early pt fills
                    # are not bandwidth-shared (rings interleave packets)
                    g.wait_ge(sem_pt[(p - 2) % PTBUF], _pt_thr(p - 2))
                g.dma_start(pt[p][:, :], pt_d[p]).then_inc(sem_pt[p], 16)
            for i in range(NI):
                p = i + PTBUF
                if p < NB:
                    g.wait_ge(sem_mm3, i + 1)
                    g.dma_start(
                        pt[p % PTBUF][:, :], pt_d[p]
                    ).then_inc(sem_pt[p % PTBUF], 16)

        @block.tensor
        def _(te):
            # ---- HAM warm-up: garbage matmuls while the startup DMAs run.
            # Data is whatever is in SBUF; aps[0][0] is overwritten by the
            # first real mm1 (start=True) before anything reads it. ----
            for _ in range(NWARM_BIG):
                nc.tensor.matmul(
                    aps[0][0][:, :],
                    h0f[:, 0:128],
                    pt[0][:, 2 : 2 + N],
                    start=True,
                    stop=True,
                    skip_group_check=True,
                )
            for _ in range(NWARM_SMALL):
                nc.tensor.matmul(
                    aps[0][0][:, 0:128],
                    h0f[:, 0:128],
                    pt[0][:, 2:130],
                    start=True,
                    stop=True,
                    skip_group_check=True,
                )
            te.wait_ge(sem_h0f, 32)
            for i in range(NI):
                if i == 1:
                    te.wait_ge(sem_w2, 32)
                if i == 0:
                    te.wait_ge(sem_pt[0], _pt_thr(0))

                # ---- mm1(i): A_T chunks, bf16 N=384; chunk TS[i] comes from
                # the DVE-patched scratch (H0 row g += W1r) ----
                if i < NB:
                    ptt = pt[i % PTBUF]
                    for hc in range(2):
                        for t in range(3):
                            if t == TS[i]:
                                lhsT = ptt[
                                    :, 3 * W + hc * 128 : 3 * W + hc * 128 + 128
                                ]
                            else:
                                lhsT = h0f[
                                    :, t * HID + hc * 128 : t * HID + hc * 128 + 128
                                ]
                            mm = nc.tensor.matmul(
                                aps[i % 2][hc][:, :],
                                lhsT,
                                ptt[:, t * W + 2 : t * W + 2 + N],
                                start=(t == 0),
                                stop=(t == 2),
                                skip_group_check=True,
                            )
                        # per-hc inc so ACT can start relu(hc0) while the
                        # hc1 matmuls still stream
                        mm.then_inc(sem_mm1, 1)

                # ---- mm4(i-3): S row into h3ps[k%2] partition 0; runs
                # right after mm1 so the ACT S-row copy starts ~a full stage
                # earlier, giving the h3ps-alias guard 3 iterations of slack ----
                k = i - 3
                if 0 <= k < NB:
                    E = ES[k]
                    te.wait_ge(sem_h3c, k + 1)
                    mm = nc.tensor.matmul(
                        h3ps[k % 2][0:1, 0:E],
                        h3sb[k % 3][:, 0:1],
                        h3sb[k % 3][:, 0:E],
                        start=True,
                        stop=True,
                    )
                    mm.then_inc(sem_mm4, 1)
                # ---- mm2(i-1): h2 = A@W2, bf16 N=128 ----
                k = i - 1
                if 0 <= k < NB:
                    te.wait_ge(sem_relu, 2 * k + 2)
                    dst = h2ps[k % 2]
                    for jc in range(3):
                        for ht in range(2):
                            mm = nc.tensor.matmul(
                                dst[:, jc * 128 : (jc + 1) * 128],
                                at[k % 3][
                                    :, ht * N + jc * 128 : ht * N + jc * 128 + 128
                                ],
                                w2f[:, ht * HID2 : (ht + 1) * HID2],
                                start=(ht == 0),
                                stop=(ht == 1),
                            )
                    if not (0 <= i - 2 < NB):
                        mm.then_inc(sem_mm2, 1)  # no mm3 rider this iter
                # ---- mm3(i-2): h3T cols 0..E (col 0 = d), N=E_s ----
                k = i - 2
                if 0 <= k < NB:
                    E = ES[k]
                    te.wait_ge(sem_h2c, k + 1)
                    if k >= 2:
                        te.wait_ge(sem_sc, k - 1)  # aliased S row was drained
                    if i + 1 < NB:
                        # next iter's pt fill, checked early: the queue digests
                        # this while mm2/mm3 stream, keeping the mm4->mm1 gap
                        # free of wait instructions
                        te.wait_ge(sem_pt[(i + 1) % PTBUF], _pt_thr(i + 1))
                    dst = h3ps[k % 2]
                    ptt = pt[k % PTBUF]
                    for t in range(3):
                        mm = nc.tensor.matmul(
                            dst[:, 0:E],
                            h2sb[k % 3][:, t * 128 : (t + 1) * 128],
                            ptt[:, t * W : t * W + E],
                            start=(t == 0),
                            stop=(t == 2),
                        )
                        if t == 0 and k + 1 < NB:
                            # completion implies same-iter mm2(k+1) drained
                            mm.then_inc(sem_mm2, 1)
                    mm.then_inc(sem_mm3, 1)
                if not (0 <= i - 2 < NB) and i + 1 < NB:
                    # pipeline-fill iters: mm3 block above was absent
                    te.wait_ge(sem_pt[(i + 1) % PTBUF], _pt_thr(i + 1))

        @block.vector
        def _(ve):
            for i in range(NI):
                k = i - 1
                if 0 <= k < NB:
                    if k >= 3:
                        ve.wait_ge(sem_mm3, k - 2)  # h2sb[k%3] reuse
                    ve.wait_ge(sem_mm2, k + 1)
                    nc.vector.tensor_copy(
                        h2sb[k % 3][:, :], h2ps[k % 2][:, :]
                    ).then_inc(sem_h2c, 1)
                k = i - 2
                if 0 <= k < NB:
                    if k >= 3:
                        ve.wait_ge(sem_mm4, k - 2)  # h3sb[k%3] reuse
                    ve.wait_ge(sem_mm3, k + 1)
                    nc.vector.tensor_copy(
                        h3sb[k % 3][:, 0 : ES[k]], h3ps[k % 2][:, 0 : ES[k]]
                    ).then_inc(sem_h3c, 1)

    return nc


def _get_nc() -> bass.Bass:
    if "nc" not in _NC_CACHE:
        _NC_CACHE["nc"] = _build_nc()
    return _NC_CACHE["nc"]


def kernel(z, x, partials, W1, W2):
    global LAST_RESULT
    z = np.asarray(z, dtype=np.float32)
    x = np.asarray(x, dtype=np.float32)
    partials = np.asarray(partials, dtype=np.float32)
    W1 = np.asarray(W1, dtype=np.float32)
    W2 = np.asarray(W2, dtype=np.float32)

    H0 = z[0] @ W1[:D]  # [384, 256]
    h0f = (
        np.ascontiguousarray(H0.reshape(3, 128, HID).transpose(1, 0, 2))
        .reshape(128, 3 * HID)
        .astype(ml_dtypes.bfloat16)
    )
    w2f = (
        np.ascontiguousarray(W2.reshape(2, 128, HID2).transpose(1, 0, 2))
        .reshape(128, 2 * HID2)
        .astype(ml_dtypes.bfloat16)
    )

    ptT = np.ascontiguousarray(partials.transpose(0, 2, 1))  # ptT[g,j,i]=P_g[i,j]
    ar = np.arange(N)
    prow = partials[ar, ar, :]  # [384, 384]  P_g[g, :]  (as fn of j)

    in_maps = []
    for c in range(NCORES):
        # slot s on core c handles context g = c + 8*(47-s)
        gs = np.array([c + NCORES * (NB - 1 - s) for s in range(NB)])
        aug = np.zeros((NB, 3, 128, W), dtype=ml_dtypes.bfloat16)
        aug[..., 2 : 2 + N] = ptT[gs].reshape(NB, 3, 128, N).astype(
            ml_dtypes.bfloat16
        )
        pr = prow[gs].reshape(NB, 3, 128).astype(ml_dtypes.bfloat16)
        aug[..., 0] = pr
        aug[..., 1] = pr
        aug = np.ascontiguousarray(aug.transpose(0, 2, 1, 3)).reshape(
            NB, 128, 3 * W
        )
        # patched H0 chunk per slot: row g of H1g = H0[g] + W1r, applied in
        # fp32 before the bf16 cast; shipped as the tail of the pt row
        pat = H0.reshape(3, 128, HID)[TS].copy()  # [NB, 128, HID] fp32
        pat[np.arange(NB), np.array(R0) + c, :] += W1[D]
        ptfull = np.concatenate(
            [aug, pat.astype(ml_dtypes.bfloat16)], axis=2
        )  # [NB, 128, PTW]
        in_maps.append(
            {
                "pt": np.ascontiguousarray(ptfull),
                "h0f": h0f,
                "w2f": w2f,
            }
        )

    nc = _get_nc()
    res = run_bass_kernel_spmd(
        nc,
        in_maps,
        core_ids=list(range(NCORES)),
        trace=bool(os.environ.get("KERNEL_TRACE")),
    )
    LAST_RESULT = res
    S = np.zeros((N, N), dtype=np.float32)
    for c in range(NCORES):
        rows = np.asarray(res.results[c]["outb"], np.float32).reshape(NB, W)
        for s in range(NB):
            g = c + NCORES * (NB - 1 - s)
            S[g, 0 : g + 1] = rows[s, 2 : 3 + g]
    sup = S + S.T  # S is already lower-triangular (diag doubled, as in TF)
    sup = sup * np.float32(0.5)
    return (x + sup).astype(np.float32)
